# revision 51
# baseline (speedup 1.0000x reference)
"""nn_Block_SpeGroup — Bass/Tile kernel for 8 TRN2 NeuronCores.

Data-parallel over batch: 4 samples per core, all params replicated.
Per-core dataflow (validated in layout_sim.py):
  PE-transpose x -> in_proj -> fused conv+SiLU (+ SE gate) in [c', tok] layout
  (c' = direction-major channel permutation, folded into weights host-side)
  -> rearrange via DRAM staging into per-direction scan tiles [(b,i), t=1024]
  -> direction-order fixups as single whole-tile strided copies (involutions)
  -> x_dbl / dt projections (PE) -> softplus (ACT)
  -> 16x per direction: e=exp(A_n*delta) (ACT), s=du*Bbc (DVE),
     h=tensor_tensor_scan(e,s) (DVE), prod=h*Cbc (DVE),
     y accumulated in PSUM via identity matmuls (PE)
  -> inverse fixups -> un-rearrange -> f2 gate, LayerNorm (ones-matmul stats),
     z gate, out_proj, PE-transpose back to token-major.
B/C broadcasts over the 32 rows ride step-0 replicated SBUF->SBUF DMAs.
"""
import numpy as np
from contextlib import ExitStack

B, H, W, DIM = 32, 32, 32, 128
K, N, R = 4, 16, 2
EPS = 1e-5
NCORES = 8
BSH = B // NCORES            # 4 samples per core
TOK = BSH * H * W            # 4096
T = 1024                     # scan length

PERM = np.array([4 * m + kt for kt in range(4) for m in range(32)])

LAST_HW_EXEC_NS = None

_f = np.arange(T)
_SWAP = (_f % 32) * 32 + (_f // 32)
_REV = 1023 - _f
_REVSWAP = 1023 - ((_f % 32) * 32 + (_f // 32))
FIXES = [None, "swap", "rev", "revswap"]
UNFIXES = ["swap", None, "revswap", "rev"]
FIX_IDX = {"swap": _SWAP, "rev": _REV, "revswap": _REVSWAP}


def _prep_weights(inp):
    p = PERM
    w = {}
    import ml_dtypes as _mld
    w["W_in_xx_T"] = np.ascontiguousarray(inp["in_proj_w"][:DIM][p].T).astype(_mld.bfloat16)
    w["W_in_z_T"] = np.ascontiguousarray(inp["in_proj_w"][DIM:][p].T).astype(_mld.bfloat16)
    w["conv_w"] = np.ascontiguousarray(inp["conv_w"][p][:, None])
    w["conv_b"] = np.ascontiguousarray(inp["conv_b"][p][:, None])
    w["fc1_wT"] = np.ascontiguousarray((inp["fc1_w"][:, p] / (H * W)).T)
    w["fc2_wT"] = np.ascontiguousarray(inp["fc2_w"][p].T)
    # Block-diagonal projection weights (single K=128 matmul per direction):
    # BC: out[(b, c32), t] = sum_i xpw[d, 2+c, i] * xs[(b,i), t]
    # BC psum layout: row 4n+b = B_n(b), row 64+4n+b = C_n(b) so each
    # (n)-broadcast reads 4 ADJACENT partitions (DMA partition-step 1).
    bc_bd = np.zeros((128, 4 * 128), np.float32)
    dt_bd = np.zeros((128, 4 * 8), np.float32)
    for d in range(4):
        for b in range(BSH):
            for n in range(N):
                bc_bd[b * 32:(b + 1) * 32, d * 128 + 4 * n + b] = \
                    inp["x_proj_weight"][d, 2 + n, :]
                bc_bd[b * 32:(b + 1) * 32, d * 128 + 64 + 4 * n + b] = \
                    inp["x_proj_weight"][d, 2 + N + n, :]
            for r in range(R):
                dt_bd[b * 32:(b + 1) * 32, d * 8 + b * 2 + r] = \
                    inp["x_proj_weight"][d, r, :]
    w["Wxp_bc"] = bc_bd.astype(_mld.bfloat16)
    w["Wxp_dt"] = dt_bd.astype(_mld.bfloat16)
    # delta: out[(b,i), t] = sum_r dtw[d, i, r] * dts[(b*2+r), t]
    dtw_bd = np.zeros((8, 4 * 128), np.float32)
    for d in range(4):
        for b in range(BSH):
            for r in range(R):
                dtw_bd[b * 2 + r, d * 128 + b * 32:d * 128 + (b + 1) * 32] = \
                    inp["dt_projs_weight"][d, :, r]
    w["dtw_bd"] = dtw_bd.astype(_mld.bfloat16)
    # NEGATED dt bias [(b,i), d]: softplus(x) computed as -ln(sigmoid(-x)),
    # so the sigmoid stage uses scale=-1 and bias=-dt_projs_bias.
    w["dt_bias"] = -np.stack(
        [np.tile(inp["dt_projs_bias"][d], BSH) for d in range(4)], axis=1
    ).astype(np.float32)
    # |A| scales [(b,i), d*16+n]: e = exp(A*delta) = exp(|A| * ln_sigmoid)
    A_full = np.exp(inp["A_logs"]).reshape(K, 32, N)
    asc = np.zeros((128, 64), np.float32)
    for d in range(4):
        asc[:, d * 16:(d + 1) * 16] = np.tile(A_full[d], (BSH, 1))
    w["A_sc"] = asc
    # Ds diagonal matrices per direction [(b,i) x (b,i)]
    Ds_full = inp["Ds"].reshape(K, 32)
    dsd = np.zeros((128, 4 * 128), np.float32)
    for d in range(4):
        dsd[:, d * 128:(d + 1) * 128] = np.diag(np.tile(Ds_full[d], BSH))
    w["DsDiag"] = dsd.astype(_mld.bfloat16)
    w["ln_g"] = np.ascontiguousarray(inp["ln_g"][p][:, None])
    w["ln_b"] = np.ascontiguousarray(inp["ln_b"][p][:, None])
    w["W_out_T"] = np.ascontiguousarray(inp["out_proj_w"][:, p].T)
    w["I128"] = np.eye(128, dtype=np.float32)
    # B-broadcast selector: selB_n[p', p] = 1 iff p' == 4n + p//32
    ml_dtypes = _mld
    w["ones_row_b"] = np.ones((1, 128), _mld.bfloat16)
    selB = np.zeros((128, 16 * 128), np.float32)
    for n in range(N):
        for pp in range(128):
            selB[4 * n + pp // 32, n * 128 + pp] = 1.0
    w["selB"] = selB.astype(ml_dtypes.bfloat16)
    selC = np.zeros((128, 16 * 128), np.float32)
    for n in range(N):
        for pp in range(128):
            selC[64 + 4 * n + pp // 32, n * 128 + pp] = 1.0
    w["selC"] = selC.astype(ml_dtypes.bfloat16)
    w["I128b"] = np.eye(128, dtype=ml_dtypes.bfloat16)
    w["I32x4"] = np.tile(np.eye(32), (4, 1)).astype(_mld.bfloat16)
    w["ones_col"] = np.ones((128, 1), np.float32)
    w["ones_row"] = np.ones((1, 128), np.float32)

    w["eps_col"] = np.full((128, 1), EPS, np.float32)
    w["ln_g_is_one"] = bool(np.all(inp["ln_g"] == 1.0))
    w["ln_b_is_zero"] = bool(np.all(inp["ln_b"] == 0.0))
    return w


WEIGHT_SHAPES = {
    "W_in_xx_T": (128, 128), "W_in_z_T": (128, 128),
    "conv_w": (128, 1), "conv_b": (128, 1),
    "fc1_wT": (128, 4), "fc2_wT": (4, 128),
    "Wxp_bc": (128, 512), "Wxp_dt": (128, 32), "dtw_bd": (8, 512),
    "dt_bias": (128, 4),
    "A_sc": (128, 64), "DsDiag": (128, 512),
    "ln_g": (128, 1), "ln_b": (128, 1),
    "W_out_T": (128, 128), "I128": (128, 128), "ones_col": (128, 1),
    "ones_row": (1, 128), "ones_row_b": (1, 128),
    "selB": (128, 2048), "selC": (128, 2048),
    "I128b": (128, 128), "I32x4": (128, 32),
    "eps_col": (128, 1),
}

BF16_WEIGHTS = {"selB", "selC", "I128b", "ones_row_b", "W_in_xx_T", "W_in_z_T",
                "Wxp_bc", "Wxp_dt", "dtw_bd", "DsDiag", "I32x4"}


def _fix_ap(src_ap, kind):
    """Strided read AP applying the f-index involution on a [128, 1024] tile."""
    import concourse.bass as bass
    t = src_ap.tensor
    pitch = t.shape[1]
    base = src_ap.offset
    if kind == "swap":      # out[32a+b] = in[32b+a]
        return bass.AP(t, base, [[pitch, 128], [1, 32], [32, 32]])
    if kind == "rev":       # out[f] = in[1023-f]
        return bass.AP(t, base + 1023, [[pitch, 128], [-1, 1024]])
    if kind == "revswap":   # out[32a+b] = in[1023-(32b+a)]
        return bass.AP(t, base + 1023, [[pitch, 128], [-1, 32], [-32, 32]])
    raise ValueError(kind)


def build_program(ln_g_is_one, ln_b_is_zero, debug_taps=False):
    import concourse.bass as bass
    import concourse.bacc as bacc
    import concourse.tile as tile
    import concourse.mybir as mybir

    F32 = mybir.dt.float32
    BF16 = mybir.dt.bfloat16
    AF = mybir.ActivationFunctionType
    OP = mybir.AluOpType

    nc = bacc.Bacc("TRN2", target_bir_lowering=False, debug=False,
                   num_devices=NCORES)

    x_in = nc.dram_tensor("x", [TOK, DIM], F32, kind="ExternalInput").ap()
    wt = {}
    for name, shape in WEIGHT_SHAPES.items():
        dt_ = BF16 if name in BF16_WEIGHTS else F32
        wt[name] = nc.dram_tensor(name, list(shape), dt_, kind="ExternalInput").ap()
    out_ext = nc.dram_tensor("out", [TOK, DIM], F32, kind="ExternalOutput").ap()

    taps = {}
    def tap(name, shape):
        if debug_taps and name not in taps:
            taps[name] = nc.dram_tensor(f"tap_{name}", list(shape), F32,
                                        kind="ExternalOutput").ap()
        return taps.get(name)

    stage_in = nc.dram_tensor("stage_in", [4, 128, T], F32)    # [d][(b,i)][(m,j)]
    stage_out = nc.dram_tensor("stage_out", [4, 128, T], F32)
    stat_scr = nc.dram_tensor("stat_scr", [4, TOK], F32)

    with tile.TileContext(nc) as tc, ExitStack() as ctx:
        wpool = ctx.enter_context(tc.tile_pool(name="wts", bufs=1))
        big = ctx.enter_context(tc.tile_pool(name="big", bufs=1))
        work = ctx.enter_context(tc.tile_pool(name="work", bufs=1))
        pipe = ctx.enter_context(tc.tile_pool(name="pipe", bufs=2))
        scan_p = ctx.enter_context(tc.tile_pool(name="scan", bufs=2))
        bc_p = ctx.enter_context(tc.tile_pool(name="bc", bufs=2))
        psA = ctx.enter_context(tc.tile_pool(name="psA", bufs=1, space="PSUM"))
        psY = ctx.enter_context(tc.tile_pool(name="psY", bufs=1, space="PSUM"))

        # ---- Phase A: load + transpose + in_proj + conv/silu + SE ----
        # x loaded flat: partition p holds tokens [32p, 32p+32) densely
        # (128 big descriptors instead of 4096 small ones).
        xflat = big.tile([128, TOK], F32, tag="xc", name="xflat")
        nc.sync.dma_start(xflat[:], x_in.rearrange("(p r) c -> p (r c)", p=128))

        w_sb = {}
        wnames = [n for n in WEIGHT_SHAPES if n not in ("selB", "selC")] + ["selB", "selC"]
        for i, name in enumerate(wnames):
            shape = WEIGHT_SHAPES[name]
            dt_ = BF16 if name in BF16_WEIGHTS else F32
            w_sb[name] = wpool.tile(list(shape), dt_, name=f"w_{name}")
            eng = nc.sync if i % 2 == 0 else nc.scalar
            eng.dma_start(w_sb[name][:], wt[name])

        xT = big.tile([128, TOK], BF16)          # [din, tok]
        xTap = xT[:]
        xTp = xTap.tensor.shape[1]
        for r2 in range(16):                     # transpose token-slice pairs
            ps_t = psA.tile([128, 256], F32, tag="mm", name="ps_t", bufs=2)
            for q in range(2):
                r0 = r2 * 2 + q
                nc.tensor.transpose(ps_t[:, q * 128:(q + 1) * 128],
                                    xflat[:, r0 * 128:(r0 + 1) * 128],
                                    w_sb["I128"][:])
            # xT[c, 32p + r0] <- ps_t[c, (q, p)]
            oap = bass.AP(xTap.tensor, xTap.offset + r2 * 2,
                          [[xTp, 128], [1, 2], [32, 128]])
            nc.scalar.copy(oap, ps_t[:])

        xc = big.tile([128, TOK], BF16)          # conv+silu output, [c', tok]
        zs = big.tile([128, TOK], F32)          # silu(z)
        zz8 = work.tile([128, 8], F32, tag="zz8")
        for g in range(8):
            sl = slice(g * 512, (g + 1) * 512)
            ps_xx = psA.tile([128, 512], F32, tag="mm", name="ps_xx", bufs=2)
            nc.tensor.matmul(ps_xx[:], w_sb["W_in_xx_T"][:],
                             xT[:, sl], start=True, stop=True)
            cv = work.tile([128, 512], F32, tag="cv", bufs=2)
            nc.scalar.activation(cv[:], ps_xx[:], AF.Identity,
                                 bias=w_sb["conv_b"][:, 0:1],
                                 scale=w_sb["conv_w"][:, 0:1])
            sg = work.tile([128, 512], F32, tag="sg", bufs=2)
            nc.scalar.activation(sg[:], cv[:], AF.Sigmoid)
            nc.vector.scalar_tensor_tensor(xc[:, sl], cv[:], 1.0, sg[:],
                                           OP.mult, OP.mult,
                                           accum_out=zz8[:, g:g + 1])
            ps_z = psA.tile([128, 512], F32, tag="mm", name="ps_z", bufs=2)
            nc.tensor.matmul(ps_z[:], w_sb["W_in_z_T"][:],
                             xT[:, sl], start=True, stop=True)
            sgz = work.tile([128, 512], F32, tag="sgz", bufs=2)
            nc.scalar.activation(sgz[:], ps_z[:], AF.Sigmoid)
            nc.vector.tensor_mul(zs[:, sl], ps_z[:], sgz[:])

        # SE gate: zz4 = pairwise sums; f2 = sigmoid(fc2 @ relu(fc1 @ zz/HW))
        zz4 = work.tile([128, 4], F32, tag="zz4")
        zz8ap = zz8[:]
        ev = bass.AP(zz8ap.tensor, zz8ap.offset, [[zz8ap.tensor.shape[1], 128], [2, 4]])
        od = bass.AP(zz8ap.tensor, zz8ap.offset + 1, [[zz8ap.tensor.shape[1], 128], [2, 4]])
        nc.vector.tensor_add(zz4[:], ev, od)
        ps_f1 = psA.tile([4, 4], F32, tag="mm", name="ps_f1", bufs=2)
        nc.tensor.matmul(ps_f1[:], w_sb["fc1_wT"][:], zz4[:], start=True, stop=True)
        f1 = work.tile([4, 4], F32, tag="f1")
        nc.scalar.activation(f1[:], ps_f1[:], AF.Relu)
        ps_f2 = psA.tile([128, 4], F32, tag="mm", name="ps_f2", bufs=2)
        nc.tensor.matmul(ps_f2[:], w_sb["fc2_wT"][:], f1[:], start=True, stop=True)
        f2 = work.tile([128, 4], F32, tag="f2")
        nc.scalar.activation(f2[:], ps_f2[:], AF.Sigmoid)

        if debug_taps:
            nc.sync.dma_start(tap("xc", (128, TOK)), xc[:])
            nc.sync.dma_start(tap("zs", (128, TOK)), zs[:])
            nc.sync.dma_start(tap("f2", (128, 4)), f2[:])

        # ---- Phase B: PE-transpose rearrange xc -> per-direction scan tiles
        # xs_d [(b,i), f] with the direction's scan order fused into the
        # PSUM->SBUF copy APs. Raw transpose output is (j, m)-major.
        # input fixups (j,m)-tiles: d0 swap, d1 none, d2 revswap, d3 rev.
        xcap = xc[:]
        # matmul operands cannot start at partition 96: copy d=3's block down
        xc3 = big.tile([32, TOK], BF16, tag="xc3", name="xc3")
        nc.vector.tensor_copy(xc3[:], xc[96:128, :])
        xc3ap = xc3[:]
        xs_t = []
        for d in range(4):
            xs = big.tile([128, T], BF16, tag=f"xs{d}", name=f"xs{d}")
            xsap = xs[:]
            xp_ = xsap.tensor.shape[1]
            for jh in range(2):
                ps_x = psA.tile([128, 512], BF16, tag="mm", name="ps_x", bufs=2)
                for jl in range(16):
                    jj = jh * 16 + jl
                    if d < 3:
                        colap = bass.AP(xcap.tensor,
                                        xcap.offset + d * 32 * xcap.tensor.shape[1] + jj,
                                        [[xcap.tensor.shape[1], 32], [32, 128]])
                        ident = w_sb["I32x4"][d * 32:(d + 1) * 32, :]
                    else:
                        colap = bass.AP(xc3ap.tensor, xc3ap.offset + jj,
                                        [[xc3ap.tensor.shape[1], 32], [32, 128]])
                        ident = w_sb["I32x4"][0:32, :]
                    nc.tensor.transpose(ps_x[:, jl * 32:(jl + 1) * 32], colap,
                                        ident)
                if d == 0:      # write at 32*mm + jj
                    oap = bass.AP(xsap.tensor, xsap.offset + jh * 16,
                                  [[xp_, 128], [1, 16], [32, 32]])
                elif d == 1:    # contiguous
                    oap = xs[:, jh * 512:(jh + 1) * 512]
                elif d == 2:    # write at 1023-(32*mm+jj)
                    oap = bass.AP(xsap.tensor, xsap.offset + 1023 - jh * 16,
                                  [[xp_, 128], [-1, 16], [-32, 32]])
                else:           # write at 1023-(32*jj+mm)
                    oap = bass.AP(xsap.tensor, xsap.offset + 1023 - jh * 512,
                                  [[xp_, 128], [-32, 16], [-1, 32]])
                nc.scalar.copy(oap, ps_x[:])
            xs_t.append(xs)

        if debug_taps:
            for d in range(4):
                nc.sync.dma_start(tap(f"xs{d}", (128, T)), xs_t[d][:])

        # ---- Phase C: per-direction projections + scan ----
        # n-loop runs in t-halves with single-bank PSUM tiles (4-deep
        # rotation) so PE selector matmuls decouple from DVE consumers;
        # the scan chains across halves via initial=h0[:, -1:].
        yy = big.tile([128, TOK], F32, tag="xc", name="yy")

        def emit_frontend(d):
            xs = xs_t[d]
            bc = pipe.tile([128, T], BF16, tag="bc", bufs=2, name=f"bc{d}")
            dts = work.tile([8, T], BF16, tag="dts", bufs=2)
            for hh in range(2):
                sl = slice(hh * 512, (hh + 1) * 512)
                ps_bc = psA.tile([128, 512], F32, tag="loop", name="ps_bc", bufs=4)
                nc.tensor.matmul(ps_bc[:],
                                 w_sb["Wxp_bc"][:, d * 128:(d + 1) * 128],
                                 xs[:, sl], start=True, stop=True)
                nc.scalar.copy(bc[:, sl], ps_bc[:])
                ps_dt = psA.tile([8, 512], F32, tag="loop", name="ps_dt", bufs=4)
                nc.tensor.matmul(ps_dt[:],
                                 w_sb["Wxp_dt"][:, d * 8:(d + 1) * 8],
                                 xs[:, sl], start=True, stop=True)
                nc.scalar.copy(dts[:, sl], ps_dt[:])
            lns = pipe.tile([128, T], F32, tag="lns", bufs=2, name=f"lns{d}")
            for hh in range(2):
                sl = slice(hh * 512, (hh + 1) * 512)
                ps_delta = psA.tile([128, 512], F32, tag="loop", name="ps_delta", bufs=4)
                nc.tensor.matmul(ps_delta[:],
                                 w_sb["dtw_bd"][:, d * 128:(d + 1) * 128],
                                 dts[:, sl], start=True, stop=True)
                nc.scalar.activation(lns[:, sl], ps_delta[:], AF.Sigmoid,
                                     bias=w_sb["dt_bias"][:, d:d + 1], scale=-1.0)
            nc.scalar.activation(lns[:], lns[:], AF.Ln)
            du = pipe.tile([128, T], BF16, tag="du", bufs=2, name=f"du{d}")
            nc.vector.scalar_tensor_tensor(du[:], lns[:], -1.0, xs[:],
                                           OP.mult, OP.mult)
            if debug_taps:
                nc.sync.dma_start(tap(f"bc{d}", (128, T)), bc[:])
                nc.sync.dma_start(tap(f"lns{d}", (128, T)), lns[:])
                nc.sync.dma_start(tap(f"du{d}", (128, T)), du[:])
            return bc, lns, du

        fe = emit_frontend(0)
        for d in range(4):
            xs = xs_t[d]
            bc, lns, du = fe
            if d < 3:
                fe_next = emit_frontend(d + 1)
            ps_yh = [psY.tile([128, 512], F32, tag="ps_y", bufs=2,
                              name=f"ps_y{d}_{hh}") for hh in range(2)]
            bcap = bc[:]
            for n in range(N):
                e = scan_p.tile([128, T], F32, tag="e")
                nc.scalar.activation(e[:], lns[:], AF.Exp,
                                     scale=w_sb["A_sc"][:, d * 16 + n:d * 16 + n + 1])
                h_half = []
                for hh in range(2):
                    sl = slice(hh * 512, (hh + 1) * 512)
                    ps_bb = psA.tile([128, 512], F32, tag="loop", name="ps_bb", bufs=4)
                    nc.tensor.matmul(ps_bb[:],
                                     w_sb["selB"][:, n * 128:(n + 1) * 128],
                                     bcap[:, sl], start=True, stop=True)
                    s = scan_p.tile([128, 512], F32, tag="s", bufs=3)
                    nc.vector.tensor_mul(s[:], du[:, sl], ps_bb[:])
                    h = scan_p.tile([128, 512], F32, tag="h", bufs=3)
                    init = 0.0 if hh == 0 else h_half[0][:, 511:512]
                    nc.vector.tensor_tensor_scan(h[:], e[:, sl], s[:], init,
                                                 OP.mult, OP.add)
                    h_half.append(h)
                    ps_cc = psA.tile([128, 512], F32, tag="loop", name="ps_cc", bufs=4)
                    nc.tensor.matmul(ps_cc[:],
                                     w_sb["selC"][:, n * 128:(n + 1) * 128],
                                     bcap[:, sl], start=True, stop=True)
                    cc = bc_p.tile([128, 512], BF16, tag="cc", bufs=4)
                    nc.scalar.copy(cc[:], ps_cc[:])
                    prod = scan_p.tile([128, 512], BF16, tag="prod", bufs=3)
                    nc.gpsimd.tensor_mul(prod[:], h[:], cc[:])
                    nc.tensor.matmul(ps_yh[hh][:], w_sb["I128b"][:], prod[:],
                                     start=(n == 0), stop=False)
                    if debug_taps and n == 0 and hh == 0:
                        nc.sync.dma_start(tap(f"e{d}", (128, T)), e[:])
                        nc.sync.dma_start(tap(f"s{d}", (128, 512)), s[:])
                        nc.sync.dma_start(tap(f"h{d}", (128, 512)), h[:])
            for hh in range(2):
                sl = slice(hh * 512, (hh + 1) * 512)
                nc.tensor.matmul(ps_yh[hh][:],
                                 w_sb["DsDiag"][:, d * 128:(d + 1) * 128],
                                 xs[:, sl], start=False, stop=True)

            # y_sb = ps_y with inverse fixup fused into the copy AP:
            # out-fixups (to (j,m)-layout): d0 none, d1 swap, d2 rev, d3 revswap
            y_sb = work.tile([128, T], F32, tag="y_sb", bufs=2)
            ysap = y_sb[:]
            yp_ = ysap.tensor.shape[1]
            for hh in range(2):
                # write region of source half hh under the direction fixup
                if d == 0:
                    uap = y_sb[:, hh * 512:(hh + 1) * 512]
                elif d == 1:   # out[32b+a] for a = hh*16..+16 local
                    uap = bass.AP(ysap.tensor, ysap.offset + hh * 16,
                                  [[yp_, 128], [1, 16], [32, 32]])
                elif d == 2:   # out[1023-f]
                    uap = bass.AP(ysap.tensor, ysap.offset + 1023 - hh * 512,
                                  [[yp_, 128], [-1, 512]])
                else:          # out[1023-(32b+a)]
                    uap = bass.AP(ysap.tensor, ysap.offset + 1023 - hh * 16,
                                  [[yp_, 128], [-1, 16], [-32, 32]])
                nc.scalar.copy(uap, ps_yh[hh][:])
            if debug_taps:
                nc.sync.dma_start(tap(f"y{d}", (128, T)), y_sb[:])

            # inverse transposes: u_d [(b,i), (j,m)] -> yy [c'=d*32+m, (bi, j)]
            yyap = yy[:]
            yyp = yyap.tensor.shape[1]
            for jq in range(8):
                ps_u = psA.tile([32, 512], F32, tag="mm", name="ps_u", bufs=2)
                for q in range(4):
                    jj = jq * 4 + q
                    nc.tensor.transpose(ps_u[:, q * 128:(q + 1) * 128],
                                        y_sb[:, jj * 32:(jj + 1) * 32],
                                        w_sb["I128"][:])
                oap = bass.AP(yyap.tensor,
                              yyap.offset + d * 32 * yyp + jq * 4,
                              [[yyp, 32], [1, 4], [32, 128]])
                nc.scalar.copy(oap, ps_u[:])
            if d < 3:
                fe = fe_next

        if debug_taps:
            nc.sync.dma_start(tap("yy", (128, TOK)), yy[:])

        # ---- Phase D: epilogue ----
        v = big.tile([128, TOK], F32, tag="xT", name="v")
        for b in range(BSH):
            sl = slice(b * 1024, (b + 1) * 1024)
            nc.vector.tensor_scalar_mul(v[:, sl], yy[:, sl], f2[:, b:b + 1])
        if debug_taps:
            nc.sync.dma_start(tap("yy", (128, TOK)), yy[:])
            nc.sync.dma_start(tap("v", (128, TOK)), v[:])
        s1row = work.tile([1, TOK], BF16, tag="s1row")
        s2row = work.tile([1, TOK], BF16, tag="s2row")
        for g in range(8):
            sl = slice(g * 512, (g + 1) * 512)
            sqg = work.tile([128, 512], F32, tag="sqg", bufs=2)
            nc.scalar.activation(sqg[:], v[:, sl], AF.Square)
            ps_st = psA.tile([33, 512], F32, tag="mm", name="ps_st", bufs=2)
            nc.tensor.matmul(ps_st[0:1, :], w_sb["ones_col"][:], v[:, sl],
                             start=True, stop=True)
            nc.tensor.matmul(ps_st[32:33, :], w_sb["ones_col"][:], sqg[:],
                             start=True, stop=True)
            nc.scalar.copy(s1row[:, sl], ps_st[0:1, :])
            nc.scalar.copy(s2row[:, sl], ps_st[32:33, :])
        # row-space stat math (in-place): mu = s1/128; istd = rsqrt(var+eps)
        mu_row = s1row
        nc.vector.tensor_scalar_mul(mu_row[:], mu_row[:], 1.0 / DIM)
        msq = work.tile([1, TOK], BF16, tag="msq")
        nc.scalar.activation(msq[:], mu_row[:], AF.Square)
        var_row = work.tile([1, TOK], BF16, tag="var_row")
        nc.vector.scalar_tensor_tensor(var_row[:], s2row[:], 1.0 / DIM, msq[:],
                                       OP.mult, OP.subtract)
        sd_row = work.tile([1, TOK], BF16, tag="sd_row")
        nc.scalar.activation(sd_row[:], var_row[:], AF.Sqrt,
                             bias=w_sb["eps_col"][0:1, 0:1])
        istd_row = work.tile([1, TOK], BF16, tag="s2row", name="istd_row")
        with nc.allow_low_precision(reason="istd bf16 ok within 2e-2 gate"):
            nc.vector.reciprocal(istd_row[:], sd_row[:])

        # LN apply (in-place on v) + z gate + out_proj + flat re-tokenized store
        o_all = big.tile([128, TOK], BF16, tag="o_all", name="o_all")
        for g in range(8):
            sl = slice(g * 512, (g + 1) * 512)
            ps_mb = psA.tile([128, 512], F32, tag="mm", name="ps_mb", bufs=2)
            nc.tensor.matmul(ps_mb[:], w_sb["ones_row_b"][:], mu_row[:, sl],
                             start=True, stop=True)
            nc.vector.tensor_sub(v[:, sl], v[:, sl], ps_mb[:])
            ps_ib = psA.tile([128, 512], F32, tag="mm", name="ps_ib", bufs=2)
            nc.tensor.matmul(ps_ib[:], w_sb["ones_row_b"][:], istd_row[:, sl],
                             start=True, stop=True)
            if ln_g_is_one:
                nc.vector.tensor_mul(v[:, sl], v[:, sl], ps_ib[:])
            else:
                nc.vector.scalar_tensor_tensor(v[:, sl], v[:, sl],
                                               w_sb["ln_g"][:, 0:1], ps_ib[:],
                                               OP.mult, OP.mult)
            if not ln_b_is_zero:
                nc.vector.tensor_scalar_add(v[:, sl], v[:, sl], w_sb["ln_b"][:, 0:1])
            nc.vector.tensor_mul(v[:, sl], v[:, sl], zs[:, sl])
            ps_o = psA.tile([128, 512], F32, tag="mm", name="ps_o", bufs=2)
            nc.tensor.matmul(ps_o[:], w_sb["W_out_T"][:], v[:, sl],
                             start=True, stop=True)
            nc.scalar.copy(o_all[:, sl], ps_o[:])
        # re-tokenize: oflat[p, (r, c)] = out[tok=32p+r, c], then one dense DMA
        oflat = big.tile([128, TOK], F32, tag="xc", name="oflat")
        oallap = o_all[:]
        oap_p = oallap.tensor.shape[1]
        ofap = oflat[:]
        for r2 in range(16):
            ps_ot = psA.tile([128, 256], BF16, tag="mm", name="ps_ot", bufs=2)
            for q in range(2):
                r0 = r2 * 2 + q
                colap = bass.AP(oallap.tensor, oallap.offset + r0,
                                [[oap_p, 128], [32, 128]])
                nc.tensor.transpose(ps_ot[:, q * 128:(q + 1) * 128], colap,
                                    w_sb["I128b"][:])
            # ps_ot [tokp(128), (q, c)] -> oflat[p, (r2*2+q)*128 + c]
            nc.scalar.copy(oflat[:, r2 * 256:(r2 + 1) * 256], ps_ot[:])
        nc.sync.dma_start(out_ext.rearrange("(p r) c -> p (r c)", p=128), ofap)

    nc.compile()
    return nc


_PROG = None


def _get_program(w):
    global _PROG
    if _PROG is None:
        _PROG = build_program(w["ln_g_is_one"], w["ln_b_is_zero"])
    return _PROG


_HOOK_INSTALLED = False


def _install_ntff_hook():
    """Provide antenv.axon_hooks (absent in this image) so
    run_bass_kernel_spmd(trace=True) can capture NTFF profiles via the
    axon PJRT .so. Mirrors trn_boot._ntff_profile_via_ctypes."""
    global _HOOK_INSTALLED
    if _HOOK_INSTALLED:
        return
    _HOOK_INSTALLED = True
    import sys, types, ctypes, contextlib, os
    so_path = "/opt/axon/libaxon_pjrt.so"
    if "antenv.axon_hooks" in sys.modules or not os.path.exists(so_path):
        return
    try:
        lib = ctypes.CDLL(so_path)
        if not hasattr(lib, "axon_start_nrt_profile"):
            return
        lib.axon_start_nrt_profile.argtypes = [ctypes.POINTER(ctypes.c_int64),
                                               ctypes.c_size_t]
        lib.axon_start_nrt_profile.restype = ctypes.c_int64
        lib.axon_stop_nrt_profile.argtypes = [ctypes.c_char_p]
        lib.axon_stop_nrt_profile.restype = ctypes.c_int64
    except OSError:
        return

    @contextlib.contextmanager
    def _hook(output_dir, device_ids):
        import jax
        jax.devices()
        if device_ids:
            ids = (ctypes.c_int64 * len(device_ids))(*device_ids)
            rc = lib.axon_start_nrt_profile(ids, len(device_ids))
        else:
            rc = lib.axon_start_nrt_profile(None, 0)
        if rc != 0:
            raise RuntimeError(f"axon_start_nrt_profile rc={rc}")
        try:
            yield
        finally:
            n = lib.axon_stop_nrt_profile(str(output_dir).encode())
            print(f"ntff profile: {n} file(s) -> {output_dir}", file=sys.stderr)

    mod = types.ModuleType("antenv.axon_hooks")
    _state = {"hook": _hook}
    mod.get_axon_ntff_profile_hook = lambda: _state["hook"]
    mod.set_axon_ntff_profile_hook = lambda h: _state.__setitem__("hook", h)
    sys.modules["antenv.axon_hooks"] = mod
    import antenv
    antenv.axon_hooks = mod


def kernel(**inputs):
    global LAST_HW_EXEC_NS
    import os
    inputs = {k: np.asarray(v, dtype=np.float32) for k, v in inputs.items()}
    w = _prep_weights(inputs)
    nc = _get_program(w)

    from concourse.bass_utils import run_bass_kernel_spmd

    x = inputs["x"]
    base = {name: np.ascontiguousarray(w[name]) for name in WEIGHT_SHAPES}
    in_maps = []
    for c in range(NCORES):
        m = dict(base)
        m["x"] = np.ascontiguousarray(
            x[c * BSH:(c + 1) * BSH].reshape(TOK, DIM))
        in_maps.append(m)

    trace = os.environ.get("KBENCH_TRACE", "0") == "1"
    if trace:
        _install_ntff_hook()
    res = run_bass_kernel_spmd(nc, in_maps, list(range(NCORES)), trace=trace)
    LAST_HW_EXEC_NS = res.exec_time_ns
    outs = [res.results[c]["out"].reshape(BSH, H, W, DIM) for c in range(NCORES)]
    return np.concatenate(outs, axis=0).astype(np.float32)
